# revision 1
# baseline (speedup 1.0000x reference)
"""Causal single-head attention with per-batch length masking, on 8 trn2 cores.

Problem: x[8,2048,1024] f32, Wq/Wk/Wv[1024,64] f32, lengths[8] int64.
  q,k,v = x@W*;  s = q@k^T (causal + length-pair mask, -inf);  s *= H^-0.5
  out = softmax(s) @ v          -> [8, 2048, 64] f32

Math note: for row i < len: every causal key j<=i is also valid (j < len), so
the pair-mask never bites -> plain causal softmax. For row i >= len: only the
diagonal survives -> out[i] = v[i]. So: compute pure causal attention and
blend rows >= len with v.

Sharding: data-parallel over batch, one batch element per NeuronCore.

Per-core kernel layout strategy:
  - host passes x TRANSPOSED (xT [1024,2048]) so the contraction dim (e) is on
    partitions for the q/k/v projections (pure layout prep, no host FLOPs).
  - matmuls run in fp32r (single-pass, full-rate; fp32 rounded to 11 mantissa
    bits). DRAM-fed operands are pre-rounded on host so the DMA'd bits are
    already on the fp32r grid; on-device operands are produced with
    float32r output dtype (ACT/DVE round on write).
  - qT,kT [64,2048] from one fused matmul (Wq|Wk stationary).
  - scores are computed TRANSPOSED: sT[j,i] = k_j . q_i, so softmax's exp
    output pT[j,i] is directly the stationary operand of the PV matmul
    (out^T[h,i] accumulated over j-tiles) -- no P transpose needed.
  - causal diag masking: -1e30 added into PSUM via a tiny bf16 bias-matmul.
  - two ones-columns appended to v (fp32r needs an even stationary free dim)
    make the PV matmul also produce the softmax denominators (row 64).
  - out^T chunks are PE-transposed back to [i,h], normalized by 1/denom and
    blended with v using the per-row length masks.
"""

import os
import sys

import numpy as np

try:
    import concourse.bass as bass  # noqa: F401
except ImportError:
    sys.path.insert(0, "/opt/trn_rl_repo")

import concourse.bass as bass
import concourse.mybir as mybir
import concourse.tile as tile
from concourse import bacc
from concourse.bass_utils import run_bass_kernel_spmd
from concourse.masks import make_identity, make_lower_triangular

F32 = mybir.dt.float32
F32R = mybir.dt.float32r
BF16 = mybir.dt.bfloat16

B, T, E, H = 8, 2048, 1024, 64
HP = H + 2       # v augmented with 2 ones-columns (even free dim for fp32r)
P = 128          # partitions
CH = 512         # i-chunk width
ET = E // P      # 8 e-tiles
NCH = T // CH    # 4 chunks
NIT = T // P     # 16 i-tiles
SCALE = float(H) ** -0.5

USE_F32R = True  # full-rate single-pass fp32r matmuls (vs 4-pass exact fp32)
MM_DT = F32R if USE_F32R else F32


def round_f32r(a: np.ndarray) -> np.ndarray:
    """Round fp32 to the fp32r grid (11 explicit mantissa bits, RNE)."""
    if not USE_F32R:
        return np.ascontiguousarray(a, dtype=np.float32)
    u = np.ascontiguousarray(a, dtype=np.float32).view(np.uint32)
    u = (u + np.uint32(0x7FF) + ((u >> np.uint32(12)) & np.uint32(1))) & np.uint32(
        0xFFFFF000
    )
    return u.view(np.float32)


def build_nc():
    nc = bacc.Bacc(
        "TRN2",
        target_bir_lowering=False,
        debug=False,
        num_devices=B,
    )

    xt_d = nc.dram_tensor("xt", [E, T], MM_DT, kind="ExternalInput").ap()
    wqk_d = nc.dram_tensor("wqk", [E, 2 * H], MM_DT, kind="ExternalInput").ap()
    wv_d = nc.dram_tensor("wv", [E, H], MM_DT, kind="ExternalInput").ap()
    m_d = nc.dram_tensor("m", [P, NIT], F32, kind="ExternalInput").ap()
    im_d = nc.dram_tensor("im", [P, NIT], F32, kind="ExternalInput").ap()
    out_d = nc.dram_tensor("out", [T, H], F32, kind="ExternalOutput").ap()

    with tile.TileContext(nc) as tc:
        with (
            tc.tile_pool(name="const", bufs=1) as cpool,
            tc.tile_pool(name="xt", bufs=1) as xtpool,
            tc.tile_pool(name="qk", bufs=1) as qkpool,
            tc.tile_pool(name="v", bufs=1) as vpool,
            tc.tile_pool(name="stage", bufs=3) as stpool,
            tc.tile_pool(name="pt", bufs=6) as ptpool,
            tc.tile_pool(name="blend", bufs=4) as blpool,
            tc.tile_pool(name="ps_s", bufs=3, space="PSUM") as ps_s,
            tc.tile_pool(name="ps_o", bufs=2, space="PSUM") as ps_o,
            tc.tile_pool(name="ps_m", bufs=3, space="PSUM") as ps_m,
        ):
            # ---- constants ----
            ident = cpool.tile([P, P], F32, tag="ident")
            make_identity(nc, ident[:])
            ident_b = cpool.tile([P, P], BF16, tag="ident_b")
            nc.vector.tensor_copy(ident_b[:], ident[:])
            # strict lower-triangular -1e30 (mask sT where j > i within block)
            ltri = cpool.tile([P, P], F32, tag="ltri")
            make_lower_triangular(nc, ltri[:], val=-1e30, diag=False)
            ltri_b = cpool.tile([P, P], BF16, tag="ltri_b")
            nc.vector.tensor_copy(ltri_b[:], ltri[:])
            ones2 = cpool.tile([P, HP - H], F32, tag="ones2")
            nc.gpsimd.memset(ones2[:], 1.0)

            # one batched DMA per weight tensor: [1024, F] -> [128, 8, F]
            wqk_all = cpool.tile([P, ET * 2 * H], MM_DT, tag="wqk")
            nc.sync.dma_start(
                out=wqk_all[:].rearrange("p (n f) -> p n f", f=2 * H),
                in_=wqk_d.rearrange("(n p) f -> p n f", p=P),
            )
            wqk_sb = [wqk_all[:, e * 2 * H : (e + 1) * 2 * H] for e in range(ET)]

            # persistent per-(e,chunk) xT tiles, per-chunk qT/kT, per-itile v
            xt_sb = [[None] * NCH for _ in range(ET)]
            qt_sb = [None] * NCH
            kt_sb = [None] * NCH
            v_sb = [None] * NIT

            def emit_xt_dmas(c):
                for e in range(ET):
                    xt = xtpool.tile([P, CH], MM_DT, tag=f"xt{e}_{c}")
                    nc.sync.dma_start(
                        out=xt[:],
                        in_=xt_d[e * P : (e + 1) * P, c * CH : (c + 1) * CH],
                    )
                    xt_sb[e][c] = xt

            # chunk-0 x data first (gates the first matmul), then the small
            # late-use tensors, then the remaining x chunks stream behind
            emit_xt_dmas(0)
            wv_all = cpool.tile([P, ET * H], MM_DT, tag="wv")
            nc.sync.dma_start(
                out=wv_all[:].rearrange("p (n f) -> p n f", f=H),
                in_=wv_d.rearrange("(n p) f -> p n f", p=P),
            )
            wv_sb = [wv_all[:, e * H : (e + 1) * H] for e in range(ET)]
            m_sb = cpool.tile([P, NIT], F32, tag="m")
            nc.sync.dma_start(out=m_sb[:], in_=m_d[:, :])
            im_sb = cpool.tile([P, NIT], F32, tag="im")
            nc.sync.dma_start(out=im_sb[:], in_=im_d[:, :])
            for c in range(1, NCH):
                emit_xt_dmas(c)

            po_sb = [None] * NCH

            def emit_proj(c):
                # q/k projection (fused): psum[0:64]=qT, [64:128]=kT
                pqk = ps_m.tile([P, CH], F32, tag="pm")
                for e in range(ET):
                    nc.tensor.matmul(
                        pqk[:],
                        wqk_sb[e],
                        xt_sb[e][c][:],
                        start=(e == 0),
                        stop=(e == ET - 1),
                    )
                qt = qkpool.tile([H, CH], MM_DT, tag=f"qt{c}")
                nc.vector.tensor_copy(qt[:], pqk[0:H, :])
                kt = qkpool.tile([H, CH], MM_DT, tag=f"kt{c}")
                nc.scalar.activation(
                    kt[:], pqk[H : 2 * H, :],
                    mybir.ActivationFunctionType.Copy,
                )
                qt_sb[c] = qt
                kt_sb[c] = kt

                # v projection (vT), then PE-transpose to v [t,h]
                pv = ps_m.tile([H, CH], F32, tag="pm")
                for e in range(ET):
                    nc.tensor.matmul(
                        pv[:],
                        wv_sb[e],
                        xt_sb[e][c][:],
                        start=(e == 0),
                        stop=(e == ET - 1),
                    )
                vt_st = stpool.tile([H, CH], F32, tag="vt")
                nc.scalar.activation(
                    vt_st[:], pv[:], mybir.ActivationFunctionType.Copy
                )
                for k in range(4):
                    it = c * 4 + k
                    pvt = ps_m.tile([P, H], F32, tag="pm")
                    nc.tensor.transpose(
                        pvt[:], vt_st[:, k * P : (k + 1) * P], ident[0:H, 0:H]
                    )
                    vt = vpool.tile([P, HP], MM_DT, tag=f"v{it}")
                    nc.vector.tensor_copy(vt[:, H:HP], ones2[:])
                    nc.vector.tensor_copy(vt[:, 0:H], pvt[:])
                    v_sb[it] = vt

            def emit_scores(c):
                # scores^T + exp + PV, accumulated over j-tiles
                po = ps_o.tile([HP, CH], F32, tag="po")
                po_sb[c] = po
                njt = 4 * c + 4
                for j in range(njt):
                    off = max(0, j * P - c * CH)
                    w = CH - off
                    pss = ps_s.tile([P, w], F32, tag="ps")
                    # sT[j-block, i] = kT[:,jblk]^T @ qT[:, i-range]
                    nc.tensor.matmul(
                        pss[:],
                        kt_sb[j // 4][:, (j % 4) * P : (j % 4 + 1) * P],
                        qt_sb[c][:, off:CH],
                        start=True,
                        stop=(j < 4 * c),
                    )
                    if j >= 4 * c:
                        # diag block: add -1e30 strict-lower-tri bias into
                        # the leading 128 cols (j > i positions)
                        nc.tensor.matmul(
                            pss[:, 0:P],
                            ident_b[:],
                            ltri_b[:],
                            start=False,
                            stop=True,
                        )
                    pt = ptpool.tile([P, w], MM_DT, tag="pt")
                    nc.scalar.activation(
                        pt[:], pss[:], mybir.ActivationFunctionType.Exp,
                        scale=SCALE,
                    )
                    nc.tensor.matmul(
                        po[:, off:CH],
                        v_sb[j][:],
                        pt[:],
                        start=(j == 0),
                        stop=(j == njt - 1),
                    )

            def emit_out_copy(c):
                ot = stpool.tile([HP, CH], F32, tag="ot")
                nc.vector.tensor_copy(ot[:], po_sb[c][:])
                return ot

            def emit_out_rest(c, ot):
                # normalize, blend with v (length mask), store
                ob = blpool.tile([P, 4 * H], F32, tag="ob")
                for k in range(4):
                    it = c * 4 + k
                    pot = ps_m.tile([P, HP], F32, tag="pm")
                    nc.tensor.transpose(
                        pot[:],
                        ot[:, k * P : (k + 1) * P],
                        ident[0:HP, 0:HP],
                    )
                    recip = blpool.tile([P, 1], F32, tag="recip")
                    nc.vector.reciprocal(recip[:], pot[:, H : H + 1])
                    rm = blpool.tile([P, 1], F32, tag="rm")
                    nc.vector.tensor_mul(rm[:], recip[:], m_sb[:, it : it + 1])
                    t1 = blpool.tile([P, H], F32, tag="t1")
                    nc.vector.tensor_scalar_mul(t1[:], pot[:, 0:H], rm[:])
                    # ob = (v * im) + t1   (one fused op)
                    nc.vector.scalar_tensor_tensor(
                        ob[:, k * H : (k + 1) * H],
                        v_sb[it][:, 0:H].bitcast(F32),
                        im_sb[:, it : it + 1],
                        t1[:],
                        op0=mybir.AluOpType.mult,
                        op1=mybir.AluOpType.add,
                    )
                nc.sync.dma_start(
                    out=out_d.rearrange("(n p) h -> p n h", p=P)[
                        :, c * 4 : (c + 1) * 4, :
                    ],
                    in_=ob[:].rearrange("p (n h) -> p n h", h=H),
                )

            # software pipeline: the (c-1) output path is emitted AFTER the
            # chunk-c projection so PE never stalls on the DVE/ACT copies
            ot_prev = None
            for c in range(NCH):
                emit_proj(c)
                if ot_prev is not None:
                    emit_out_rest(c - 1, ot_prev)
                emit_scores(c)
                ot_prev = emit_out_copy(c)
            emit_out_rest(NCH - 1, ot_prev)

    nc.compile()
    return nc


_NC_CACHE = None


def _get_nc():
    global _NC_CACHE
    if _NC_CACHE is None:
        _NC_CACHE = build_nc()
    return _NC_CACHE


def make_in_maps(x, Wq, Wk, Wv, lengths):
    wqk = round_f32r(
        np.concatenate(
            [np.asarray(Wq, dtype=np.float32), np.asarray(Wk, dtype=np.float32)],
            axis=1,
        )
    )
    wv = round_f32r(np.asarray(Wv, dtype=np.float32))
    in_maps = []
    for b in range(B):
        xt = round_f32r(np.asarray(x[b], dtype=np.float32).T)
        mflat = (np.arange(T) < int(lengths[b])).astype(np.float32)
        m = np.ascontiguousarray(mflat.reshape(NIT, P).T)  # [128, 16]
        im = np.ascontiguousarray(1.0 - m)
        in_maps.append({"xt": xt, "wqk": wqk, "wv": wv, "m": m, "im": im})
    return in_maps


def run(x, Wq, Wk, Wv, lengths, trace=False):
    nc = _get_nc()
    in_maps = make_in_maps(x, Wq, Wk, Wv, lengths)
    res = run_bass_kernel_spmd(
        nc, in_maps, core_ids=list(range(B)), trace=trace
    )
    out = np.stack([res.results[b]["out"] for b in range(B)], axis=0)
    return out, res


def kernel(x, Wq, Wk, Wv, lengths):
    out, _ = run(x, Wq, Wk, Wv, lengths, trace=False)
    return out



# revision 2
# speedup vs baseline: 1.0522x; 1.0522x over previous
"""Causal single-head attention with per-batch length masking, on 8 trn2 cores.

Problem: x[8,2048,1024] f32, Wq/Wk/Wv[1024,64] f32, lengths[8] int64.
  q,k,v = x@W*;  s = q@k^T (causal + length-pair mask, -inf);  s *= H^-0.5
  out = softmax(s) @ v          -> [8, 2048, 64] f32

Math note: for row i < len: every causal key j<=i is also valid (j < len), so
the pair-mask never bites -> plain causal softmax. For row i >= len: only the
diagonal survives -> out[i] = v[i]. So: compute pure causal attention and
blend rows >= len with v.

Sharding: data-parallel over batch, one batch element per NeuronCore.

Per-core kernel design (v2, all-bf16):
  - host passes x TRANSPOSED and cast to bf16 (xT [1024,2048]) so the
    contraction dim (e) is on partitions for the q/k/v projections. bf16
    halves HBM traffic (4MB/core) and enables FWL fast weight loads.
  - interleaved phases per 512-column "quarter" c: projection chunk c
    (qT,kT,vT via wqk/wv-stationary matmuls), then attention quarter c
    (j-tiles 0..4c+3). This keeps the PE dense (HAM stays warm) and lets
    the scalar engine start exp early (exp is a ~15us serial floor).
  - scores computed TRANSPOSED: sT[j,i] = k_j . q_i, so the exp output
    pT[j,i] feeds the PV matmul directly (v stationary, pT moving).
  - causal diag masking: gpsimd affine_select zeroes the j>i half of the
    diagonal 128x128 block of pT after exp (no PE bias matmul).
  - two ones-columns appended to v make the PV matmul also produce the
    softmax denominators (row 64).
  - all transposes are regular bf16 matmuls with the data STATIONARY and
    a small identity MOVING (f32 is_transpose is 3-4x slower on PE).
  - per-quarter blend: poT -> bf16 -> PE transpose -> [i,h] f32, normalize
    by 1/denom, blend rows >= len with v via per-row masks, DMA out.
"""

import sys

import numpy as np

try:
    import concourse.bass as bass  # noqa: F401
except ImportError:
    sys.path.insert(0, "/opt/trn_rl_repo")

import concourse.bass as bass
import concourse.mybir as mybir
import concourse.tile as tile
from concourse import bacc
from concourse.bass_utils import run_bass_kernel_spmd
from concourse.masks import make_identity

F32 = mybir.dt.float32
BF16 = mybir.dt.bfloat16

B, T, E, H = 8, 2048, 1024, 64
HP = H + 2       # v augmented with 2 ones-columns (denominator trick)
P = 128          # partitions
CH = 512         # i-chunk width (quarter)
ET = E // P      # 8 e-tiles
NCH = T // CH    # 4 chunks
NIT = T // P     # 16 i-tiles
SCALE = float(H) ** -0.5


def build_nc():
    nc = bacc.Bacc(
        "TRN2",
        target_bir_lowering=False,
        debug=False,
        num_devices=B,
    )

    xt_d = nc.dram_tensor("xt", [E, T], BF16, kind="ExternalInput").ap()
    wqk_d = nc.dram_tensor("wqk", [E, 2 * H], BF16, kind="ExternalInput").ap()
    wv_d = nc.dram_tensor("wv", [E, H], BF16, kind="ExternalInput").ap()
    m_d = nc.dram_tensor("m", [P, NIT], F32, kind="ExternalInput").ap()
    im_d = nc.dram_tensor("im", [P, NIT], F32, kind="ExternalInput").ap()
    out_d = nc.dram_tensor("out", [T, H], F32, kind="ExternalOutput").ap()

    with tile.TileContext(nc) as tc:
        with (
            tc.tile_pool(name="const", bufs=1) as cpool,
            tc.tile_pool(name="xt", bufs=1) as xtpool,
            tc.tile_pool(name="qk", bufs=1) as qkpool,
            tc.tile_pool(name="v", bufs=1) as vpool,
            tc.tile_pool(name="pt", bufs=4) as ptpool,
            tc.tile_pool(name="blend", bufs=4) as blpool,
            tc.tile_pool(name="ob", bufs=2) as obpool,
            tc.tile_pool(name="pp", bufs=2, space="PSUM") as pp,
            tc.tile_pool(name="ps_t", bufs=2, space="PSUM") as ps_t,
            tc.tile_pool(name="ps_s", bufs=2, space="PSUM") as ps_s,
            tc.tile_pool(name="ps_po", bufs=2, space="PSUM") as ps_po,
        ):
            # ---- constants ----
            ident = cpool.tile([HP, HP], F32, tag="ident")
            make_identity(nc, ident[:])
            ident_b = cpool.tile([HP, HP], BF16, tag="ident_b")
            nc.vector.tensor_copy(ident_b[:], ident[:])
            warm = cpool.tile([P, 1], F32, tag="warm")
            nc.gpsimd.memset(warm[:], 0.0)
            # warm-up exp so the ACT table set loads during the DMA ramp
            warm2 = cpool.tile([P, 1], F32, tag="warm2")
            nc.scalar.activation(
                warm2[:], warm[:], mybir.ActivationFunctionType.Exp
            )

            # one batched DMA per weight tensor: [1024, F] -> [128, 8, F]
            wqk_all = cpool.tile([P, ET * 2 * H], BF16, tag="wqk")
            nc.sync.dma_start(
                out=wqk_all[:].rearrange("p (n f) -> p n f", f=2 * H),
                in_=wqk_d.rearrange("(n p) f -> p n f", p=P),
            )
            wqk_sb = [wqk_all[:, e * 2 * H : (e + 1) * 2 * H] for e in range(ET)]

            # persistent tiles
            xt_sb = [[None] * NCH for _ in range(ET)]
            qt_all = qkpool.tile([H, T], BF16, tag="qt")
            kt_all = qkpool.tile([H, T], BF16, tag="kt")
            vt_all = qkpool.tile([H, T], BF16, tag="vt")
            v_sb = [None] * NIT

            def emit_xt_dmas(c):
                for e in range(ET):
                    xt = xtpool.tile([P, CH], BF16, tag=f"xt{e}_{c}")
                    nc.sync.dma_start(
                        out=xt[:],
                        in_=xt_d[e * P : (e + 1) * P, c * CH : (c + 1) * CH],
                    )
                    xt_sb[e][c] = xt

            # chunk-0 x data first (gates the first matmul), then the small
            # late-use tensors, then the remaining x chunks stream behind
            emit_xt_dmas(0)
            wv_all = cpool.tile([P, ET * H], BF16, tag="wv")
            nc.sync.dma_start(
                out=wv_all[:].rearrange("p (n f) -> p n f", f=H),
                in_=wv_d.rearrange("(n p) f -> p n f", p=P),
            )
            wv_sb = [wv_all[:, e * H : (e + 1) * H] for e in range(ET)]
            m_sb = cpool.tile([P, NIT], F32, tag="m")
            nc.sync.dma_start(out=m_sb[:], in_=m_d[:, :])
            im_sb = cpool.tile([P, NIT], F32, tag="im")
            nc.sync.dma_start(out=im_sb[:], in_=im_d[:, :])
            for c in range(1, NCH):
                emit_xt_dmas(c)

            def emit_proj(c):
                # q/k projection (fused): psum[0:64]=qT, [64:128]=kT
                pqk = pp.tile([P, CH], F32, tag="pp")
                for e in range(ET):
                    nc.tensor.matmul(
                        pqk[:],
                        wqk_sb[e],
                        xt_sb[e][c][:],
                        start=(e == 0),
                        stop=(e == ET - 1),
                    )
                nc.vector.tensor_copy(
                    qt_all[:, c * CH : (c + 1) * CH], pqk[0:H, :]
                )
                nc.scalar.activation(
                    kt_all[:, c * CH : (c + 1) * CH],
                    pqk[H : 2 * H, :],
                    mybir.ActivationFunctionType.Copy,
                )

                # v projection (vT)
                pv = pp.tile([H, CH], F32, tag="pp")
                for e in range(ET):
                    nc.tensor.matmul(
                        pv[:],
                        wv_sb[e],
                        xt_sb[e][c][:],
                        start=(e == 0),
                        stop=(e == ET - 1),
                    )
                nc.vector.tensor_copy(vt_all[:, c * CH : (c + 1) * CH], pv[:])
                # transpose each 128-block: stationary vt slice, moving ident
                for k in range(4):
                    it = c * 4 + k
                    pvt = ps_t.tile([P, H], F32, tag="pst")
                    nc.tensor.matmul(
                        pvt[:],
                        vt_all[:, it * P : (it + 1) * P],
                        ident_b[0:H, 0:H],
                        start=True,
                        stop=True,
                    )
                    vtile = vpool.tile([P, HP], BF16, tag=f"v{it}")
                    nc.vector.tensor_copy(vtile[:, 0:H], pvt[:])
                    nc.gpsimd.memset(vtile[:, H:HP], 1.0)
                    v_sb[it] = vtile

            def emit_attn(c):
                # scores^T + exp + PV, j-tiles 0..4c+3, software-pipelined
                po = ps_po.tile([HP, CH], F32, tag="po")
                njt = 4 * c + 4
                pend = None

                def emit_pv(j, pt, off):
                    nc.tensor.matmul(
                        po[:, off:CH],
                        v_sb[j][:],
                        pt[:],
                        start=(j == 0),
                        stop=(j == njt - 1),
                    )

                for j in range(njt):
                    off = max(0, j * P - c * CH)
                    w = CH - off
                    pss = ps_s.tile([P, w], F32, tag="pss")
                    # sT[j-block, i] = kT[:,jblk]^T @ qT[:, i-range]
                    nc.tensor.matmul(
                        pss[:],
                        kt_all[:, j * P : (j + 1) * P],
                        qt_all[:, c * CH + off : (c + 1) * CH],
                        start=True,
                        stop=True,
                    )
                    pt = ptpool.tile([P, w], BF16, tag="pt")
                    nc.scalar.activation(
                        pt[:], pss[:], mybir.ActivationFunctionType.Exp,
                        scale=SCALE,
                    )
                    if j >= 4 * c:
                        # diag block (tile cols 0:128): zero keys below the
                        # diagonal: keep where (col - row) >= 0
                        nc.gpsimd.affine_select(
                            out=pt[:, 0:P],
                            in_=pt[:, 0:P],
                            compare_op=mybir.AluOpType.is_ge,
                            fill=0.0,
                            base=0,
                            channel_multiplier=-1,
                            pattern=[[1, P]],
                        )
                    if pend is not None:
                        emit_pv(*pend)
                    pend = (j, pt, off)
                emit_pv(*pend)
                return po

            def emit_blend(c, po):
                # poT [66,512] -> bf16 -> per-128-block transpose -> [i,h],
                # normalize by 1/denom (row 64), blend rows >= len with v
                po_b = blpool.tile([HP, CH], BF16, tag="pob")
                nc.vector.tensor_copy(po_b[:], po[:])
                ob = obpool.tile([P, 4 * H], F32, tag="ob")
                for k in range(4):
                    it = c * 4 + k
                    pot = ps_t.tile([P, HP], F32, tag="pst")
                    nc.tensor.matmul(
                        pot[:],
                        po_b[:, k * P : (k + 1) * P],
                        ident_b[:],
                        start=True,
                        stop=True,
                    )
                    recip = blpool.tile([P, 1], F32, tag="recip")
                    nc.vector.reciprocal(recip[:], pot[:, H : H + 1])
                    rm = blpool.tile([P, 1], F32, tag="rm")
                    nc.vector.tensor_mul(rm[:], recip[:], m_sb[:, it : it + 1])
                    t1 = blpool.tile([P, H], F32, tag="t1")
                    nc.vector.tensor_scalar_mul(t1[:], pot[:, 0:H], rm[:])
                    # ob = (v * im) + t1   (one fused op)
                    nc.vector.scalar_tensor_tensor(
                        ob[:, k * H : (k + 1) * H],
                        v_sb[it][:, 0:H],
                        im_sb[:, it : it + 1],
                        t1[:],
                        op0=mybir.AluOpType.mult,
                        op1=mybir.AluOpType.add,
                    )
                nc.sync.dma_start(
                    out=out_d.rearrange("(n p) h -> p n h", p=P)[
                        :, c * 4 : (c + 1) * 4, :
                    ],
                    in_=ob[:].rearrange("p (n h) -> p n h", h=H),
                )

            # main schedule: proj chunk c, blend of quarter c-1 (PE transposes
            # slot in while DVE drains), attention quarter c
            po_prev = None
            for c in range(NCH):
                emit_proj(c)
                if po_prev is not None:
                    emit_blend(c - 1, po_prev)
                po_prev = emit_attn(c)
            emit_blend(NCH - 1, po_prev)

    nc.compile()
    return nc


_NC_CACHE = None


def _get_nc():
    global _NC_CACHE
    if _NC_CACHE is None:
        _NC_CACHE = build_nc()
    return _NC_CACHE


def make_in_maps(x, Wq, Wk, Wv, lengths):
    bf16 = mybir.dt.np(BF16)
    wqk = np.concatenate(
        [np.asarray(Wq, dtype=np.float32), np.asarray(Wk, dtype=np.float32)],
        axis=1,
    ).astype(bf16)
    wv = np.asarray(Wv, dtype=np.float32).astype(bf16)
    in_maps = []
    for b in range(B):
        xt = np.ascontiguousarray(
            np.asarray(x[b], dtype=np.float32).T
        ).astype(bf16)
        mflat = (np.arange(T) < int(lengths[b])).astype(np.float32)
        m = np.ascontiguousarray(mflat.reshape(NIT, P).T)  # [128, 16]
        im = np.ascontiguousarray(1.0 - m)
        in_maps.append({"xt": xt, "wqk": wqk, "wv": wv, "m": m, "im": im})
    return in_maps


def run(x, Wq, Wk, Wv, lengths, trace=False):
    nc = _get_nc()
    in_maps = make_in_maps(x, Wq, Wk, Wv, lengths)
    res = run_bass_kernel_spmd(
        nc, in_maps, core_ids=list(range(B)), trace=trace
    )
    out = np.stack([res.results[b]["out"] for b in range(B)], axis=0)
    return out, res


def kernel(x, Wq, Wk, Wv, lengths):
    out, _ = run(x, Wq, Wk, Wv, lengths, trace=False)
    return out


# revision 3
# speedup vs baseline: 1.2317x; 1.1705x over previous
"""Causal single-head attention with per-batch length masking, on 8 trn2 cores.

Problem: x[8,2048,1024] f32, Wq/Wk/Wv[1024,64] f32, lengths[8] int64.
  q,k,v = x@W*;  s = q@k^T (causal + length-pair mask, -inf);  s *= H^-0.5
  out = softmax(s) @ v          -> [8, 2048, 64] f32

Math note: for row i < len: every causal key j<=i is also valid (j < len), so
the pair-mask never bites -> plain causal softmax. For row i >= len: only the
diagonal survives -> out[i] = v[i]. So: compute pure causal attention and
blend rows >= len with v.

Sharding: data-parallel over batch, one batch element per NeuronCore.

Per-core kernel design (v2, all-bf16, engine-balanced):
  - host passes x transposed+cast to bf16, repacked chunk-major so every DMA
    is one contiguous burst. Weights host-packed to [128, e-tiles*F] layout.
  - interleaved phases per 512-column quarter c: projection chunk c, then
    attention quarter c (j-tiles 0..4c+3). PE stays dense; scalar engine
    (exp) starts early because exp (~22us over 40 instrs) is near-critical.
  - scalar engine does ONLY exp. All psum->sbuf copies are on DVE, diag
    causal masking is gpsimd affine_select on the exp output, denominators
    via two ones-columns on v, transposes are bf16 data-stationary matmuls
    against a small identity moving operand.
  - psum pools sized so the PE can run 3 score-matmuls ahead of exp.
  - output written p-major ([128, 16, 64]) and unshuffled on host.
"""

import sys

import numpy as np

try:
    import concourse.bass as bass  # noqa: F401
except ImportError:
    sys.path.insert(0, "/opt/trn_rl_repo")

import concourse.bass as bass
import concourse.mybir as mybir
import concourse.tile as tile
from concourse import bacc
from concourse.bass_utils import run_bass_kernel_spmd
from concourse.masks import make_identity

F32 = mybir.dt.float32
BF16 = mybir.dt.bfloat16

B, T, E, H = 8, 2048, 1024, 64
HP = H + 2       # v augmented with 2 ones-columns (denominator trick)
P = 128          # partitions
CH = 512         # i-chunk width (quarter)
ET = E // P      # 8 e-tiles
NCH = T // CH    # 4 chunks
NIT = T // P     # 16 i-tiles
SCALE = float(H) ** -0.5


def build_nc():
    nc = bacc.Bacc(
        "TRN2",
        target_bir_lowering=False,
        debug=False,
        num_devices=B,
    )

    # xt repacked on host: block (c, e) of [128, 512] is contiguous
    xt_d = nc.dram_tensor("xt", [NCH * ET * P, CH], BF16, kind="ExternalInput").ap()
    # weights host-packed p-major: [128, e*F]
    wqk_d = nc.dram_tensor("wqk", [P, ET * 2 * H], BF16, kind="ExternalInput").ap()
    wv_d = nc.dram_tensor("wv", [P, ET * H], BF16, kind="ExternalInput").ap()
    m_d = nc.dram_tensor("m", [P, NIT], F32, kind="ExternalInput").ap()
    im_d = nc.dram_tensor("im", [P, NIT], F32, kind="ExternalInput").ap()
    # output p-major [128, 16, 64], host unshuffles
    out_d = nc.dram_tensor("out", [P, NIT * H], F32, kind="ExternalOutput").ap()

    with tile.TileContext(nc) as tc:
        with (
            tc.tile_pool(name="const", bufs=1) as cpool,
            tc.tile_pool(name="xt", bufs=1) as xtpool,
            tc.tile_pool(name="qk", bufs=1) as qkpool,
            tc.tile_pool(name="v", bufs=1) as vpool,
            tc.tile_pool(name="pt", bufs=6) as ptpool,
            tc.tile_pool(name="blend", bufs=4) as blpool,
            tc.tile_pool(name="ob", bufs=2) as obpool,
            tc.tile_pool(name="pp", bufs=2, space="PSUM") as pp,
            tc.tile_pool(name="ps_t", bufs=1, space="PSUM") as ps_t,
            tc.tile_pool(name="ps_s", bufs=3, space="PSUM") as ps_s,
            tc.tile_pool(name="ps_po", bufs=2, space="PSUM") as ps_po,
        ):
            # ---- constants ----
            ident = cpool.tile([HP, HP], F32, tag="ident")
            make_identity(nc, ident[:])
            ident_b = cpool.tile([HP, HP], BF16, tag="ident_b")
            nc.vector.tensor_copy(ident_b[:], ident[:])
            warm = cpool.tile([P, 1], F32, tag="warm")
            nc.gpsimd.memset(warm[:], 0.0)
            # warm-up exp so the ACT table set loads during the DMA ramp
            warm2 = cpool.tile([P, 1], F32, tag="warm2")
            nc.scalar.activation(
                warm2[:], warm[:], mybir.ActivationFunctionType.Exp
            )

            wqk_all = cpool.tile([P, ET * 2 * H], BF16, tag="wqk")
            nc.sync.dma_start(out=wqk_all[:], in_=wqk_d[:, :])
            wqk_sb = [wqk_all[:, e * 2 * H : (e + 1) * 2 * H] for e in range(ET)]

            # persistent tiles
            xt_sb = [[None] * NCH for _ in range(ET)]
            qt_all = qkpool.tile([H, T], BF16, tag="qt")
            kt_all = qkpool.tile([H, T], BF16, tag="kt")
            vt_all = qkpool.tile([H, T], BF16, tag="vt")
            v_sb = [None] * NIT

            def emit_xt_dmas(c):
                for e in range(ET):
                    xt = xtpool.tile([P, CH], BF16, tag=f"xt{e}_{c}")
                    nc.sync.dma_start(
                        out=xt[:],
                        in_=xt_d[(c * ET + e) * P : (c * ET + e + 1) * P, :],
                    )
                    xt_sb[e][c] = xt

            emit_xt_dmas(0)
            wv_all = cpool.tile([P, ET * H], BF16, tag="wv")
            nc.sync.dma_start(out=wv_all[:], in_=wv_d[:, :])
            wv_sb = [wv_all[:, e * H : (e + 1) * H] for e in range(ET)]
            m_sb = cpool.tile([P, NIT], F32, tag="m")
            nc.sync.dma_start(out=m_sb[:], in_=m_d[:, :])
            im_sb = cpool.tile([P, NIT], F32, tag="im")
            nc.sync.dma_start(out=im_sb[:], in_=im_d[:, :])
            for c in range(1, NCH):
                emit_xt_dmas(c)

            def emit_proj(c):
                # q/k projection (fused): psum[0:64]=qT, [64:128]=kT
                pqk = pp.tile([P, CH], F32, tag="pp")
                for e in range(ET):
                    nc.tensor.matmul(
                        pqk[:],
                        wqk_sb[e],
                        xt_sb[e][c][:],
                        start=(e == 0),
                        stop=(e == ET - 1),
                    )
                nc.vector.tensor_copy(
                    qt_all[:, c * CH : (c + 1) * CH], pqk[0:H, :]
                )
                nc.vector.tensor_copy(
                    kt_all[:, c * CH : (c + 1) * CH], pqk[H : 2 * H, :]
                )

                # v projection (vT)
                pv = pp.tile([H, CH], F32, tag="pp")
                for e in range(ET):
                    nc.tensor.matmul(
                        pv[:],
                        wv_sb[e],
                        xt_sb[e][c][:],
                        start=(e == 0),
                        stop=(e == ET - 1),
                    )
                nc.vector.tensor_copy(vt_all[:, c * CH : (c + 1) * CH], pv[:])
                # transpose each 128-block: stationary vt slice, moving ident
                for k in range(4):
                    it = c * 4 + k
                    pvt = ps_t.tile([P, H], F32, tag="pst")
                    nc.tensor.matmul(
                        pvt[:],
                        vt_all[:, it * P : (it + 1) * P],
                        ident_b[0:H, 0:H],
                        start=True,
                        stop=True,
                    )
                    vtile = vpool.tile([P, HP], BF16, tag=f"v{it}")
                    nc.vector.tensor_copy(vtile[:, 0:H], pvt[:])
                    nc.gpsimd.memset(vtile[:, H:HP], 1.0)
                    v_sb[it] = vtile

            def emit_attn(c):
                # scores^T + exp + PV, j-tiles 0..4c+3, software-pipelined
                po = ps_po.tile([HP, CH], F32, tag="po")
                njt = 4 * c + 4
                pend = None

                def emit_pv(j, pt, off):
                    nc.tensor.matmul(
                        po[:, off:CH],
                        v_sb[j][:],
                        pt[:],
                        start=(j == 0),
                        stop=(j == njt - 1),
                    )

                for j in range(njt):
                    off = max(0, j * P - c * CH)
                    w = CH - off
                    pss = ps_s.tile([P, w], F32, tag="pss")
                    # sT[j-block, i] = kT[:,jblk]^T @ qT[:, i-range]
                    nc.tensor.matmul(
                        pss[:],
                        kt_all[:, j * P : (j + 1) * P],
                        qt_all[:, c * CH + off : (c + 1) * CH],
                        start=True,
                        stop=True,
                    )
                    pt = ptpool.tile([P, w], BF16, tag="pt")
                    nc.scalar.activation(
                        pt[:], pss[:], mybir.ActivationFunctionType.Exp,
                        scale=SCALE,
                    )
                    if j >= 4 * c:
                        # diag block (tile cols 0:128): zero keys below the
                        # diagonal: keep where (col - row) >= 0
                        nc.gpsimd.affine_select(
                            out=pt[:, 0:P],
                            in_=pt[:, 0:P],
                            compare_op=mybir.AluOpType.is_ge,
                            fill=0.0,
                            base=0,
                            channel_multiplier=-1,
                            pattern=[[1, P]],
                        )
                    if pend is not None:
                        emit_pv(*pend)
                    pend = (j, pt, off)
                emit_pv(*pend)
                return po

            def emit_blend(c, po):
                # poT [66,512] -> bf16 -> per-128-block transpose -> [i,h],
                # normalize by 1/denom (row 64), blend rows >= len with v
                po_b = blpool.tile([HP, CH], BF16, tag="pob")
                nc.vector.tensor_copy(po_b[:], po[:])
                ob = obpool.tile([P, 4 * H], F32, tag="ob")
                for k in range(4):
                    it = c * 4 + k
                    pot = ps_t.tile([P, HP], F32, tag="pst")
                    nc.tensor.matmul(
                        pot[:],
                        po_b[:, k * P : (k + 1) * P],
                        ident_b[:],
                        start=True,
                        stop=True,
                    )
                    recip = blpool.tile([P, 1], F32, tag="recip")
                    nc.vector.reciprocal(recip[:], pot[:, H : H + 1])
                    rm = blpool.tile([P, 1], F32, tag="rm")
                    nc.vector.tensor_mul(rm[:], recip[:], m_sb[:, it : it + 1])
                    t1 = blpool.tile([P, H], F32, tag="t1")
                    nc.vector.tensor_scalar_mul(t1[:], pot[:, 0:H], rm[:])
                    # ob = (v * im) + t1   (one fused op)
                    nc.vector.scalar_tensor_tensor(
                        ob[:, k * H : (k + 1) * H],
                        v_sb[it][:, 0:H],
                        im_sb[:, it : it + 1],
                        t1[:],
                        op0=mybir.AluOpType.mult,
                        op1=mybir.AluOpType.add,
                    )
                nc.sync.dma_start(
                    out=out_d[:, c * 4 * H : (c + 1) * 4 * H],
                    in_=ob[:],
                )

            # main schedule: proj chunk c, blend of quarter c-1 (PE transposes
            # slot in while DVE drains), attention quarter c
            po_prev = None
            for c in range(NCH):
                emit_proj(c)
                if po_prev is not None:
                    emit_blend(c - 1, po_prev)
                po_prev = emit_attn(c)
            emit_blend(NCH - 1, po_prev)

    nc.compile()
    return nc


_NC_CACHE = None


def _get_nc():
    global _NC_CACHE
    if _NC_CACHE is None:
        _NC_CACHE = build_nc()
    return _NC_CACHE


def make_in_maps(x, Wq, Wk, Wv, lengths):
    bf16 = mybir.dt.np(BF16)
    wqk_f = np.concatenate(
        [np.asarray(Wq, dtype=np.float32), np.asarray(Wk, dtype=np.float32)],
        axis=1,
    )  # [E, 128]
    # pack p-major: [128, e*128] with wqk_p[p, e*F+f] = wqk_f[e*128+p, f]
    wqk = np.ascontiguousarray(
        wqk_f.reshape(ET, P, 2 * H).transpose(1, 0, 2).reshape(P, ET * 2 * H)
    ).astype(bf16)
    wv_f = np.asarray(Wv, dtype=np.float32)
    wv = np.ascontiguousarray(
        wv_f.reshape(ET, P, H).transpose(1, 0, 2).reshape(P, ET * H)
    ).astype(bf16)
    in_maps = []
    for b in range(B):
        xtb = np.asarray(x[b], dtype=np.float32).T  # [E, T]
        # repack chunk-major: block (c, e) contiguous [128, 512]
        xt = np.ascontiguousarray(
            xtb.reshape(ET, P, NCH, CH).transpose(2, 0, 1, 3).reshape(
                NCH * ET * P, CH
            )
        ).astype(bf16)
        mflat = (np.arange(T) < int(lengths[b])).astype(np.float32)
        m = np.ascontiguousarray(mflat.reshape(NIT, P).T)  # [128, 16]
        im = np.ascontiguousarray(1.0 - m)
        in_maps.append({"xt": xt, "wqk": wqk, "wv": wv, "m": m, "im": im})
    return in_maps


def run(x, Wq, Wk, Wv, lengths, trace=False):
    nc = _get_nc()
    in_maps = make_in_maps(x, Wq, Wk, Wv, lengths)
    res = run_bass_kernel_spmd(
        nc, in_maps, core_ids=list(range(B)), trace=trace
    )
    # out is p-major [128, 16*64] -> [T, H]
    out = np.stack(
        [
            np.ascontiguousarray(
                res.results[b]["out"].reshape(P, NIT, H).transpose(1, 0, 2)
            ).reshape(T, H)
            for b in range(B)
        ],
        axis=0,
    )
    return out, res


def kernel(x, Wq, Wk, Wv, lengths):
    out, _ = run(x, Wq, Wk, Wv, lengths, trace=False)
    return out
